# revision 49
# baseline (speedup 1.0000x reference)
"""Trainium2 Bass kernel for Transformer-XL style relative-position attention.

Reference computation (B=2, Tq=1024, Tkv=2048, D=1024, H=16, Dv=64):
    q/k/v/r projections, ac = (q+cb)@k^T, bd = rel_shift((q+pb)@r^T),
    softmax((ac+bd)/8) with causal-with-memory mask, ctx = attn@v,
    out = LN(ctx@Wo + query).

Sharding (Megatron-style tensor parallel over heads, 8 cores):
  - each core owns 2 heads: Wq/Wk/Wv/Wr column shards [1024,128], biases
    shard with heads; activations broadcast (feature-major on host).
  - ctx^T shards exchanged with TWO half-size AllToAlls (one per batch) so
    the batch-0 output projection + LayerNorm overlaps batch-1 attention.
  - each core ends up with full ctx^T for 128 tokens of each batch:
    out rows [0,128) = batch-0 tokens [128c,128c+128), rows [128,256) =
    batch-1 tokens [128c,128c+128).

Device-side numerics (|ctx@Wo| is ~1.4% of |query|, so the attention path
tolerates fp8):
  - projections / ctx / output projection in fp8e4 + DoubleRow perf mode
    (2 contraction planes per pass); weights pre-scaled x32 on host,
    de-scaled on the PSUM->SBUF copy.
  - scores run both heads in ONE K=128 fp8 matmul: the stationary holds a
    block-diagonal dual-head q tile (head h's features x its 64 q rows,
    zeros elsewhere) with the two heads interleaved in the M dim
    (partition 2c+h), because the PE streams moving data at HALF rate
    when the contraction uses <=64 partitions.
  - rel_shift via flat DRAM scratch (write rows, read back at offset
    1023-q with row stride Tkv-1), fp8, split per q-half and per head
    region so pass 2 of a batch starts when half of pass 1 has landed.
  - shifted bd added to the ac PSUM on DVE; causal mask via one
    affine_select with a stride -2 iota (j <= base + p//2); exp on
    ScalarE writes UNNORMALIZED fp8 attn (fp8 normal range; accum_out
    row sums are f32 exact); 1/rowsum (x16 ctx scale) folded into the
    ctx PSUM->SBUF copy via an interleaved reciprocal broadcast read.
  - attn round trip: fp8 written contiguously per head region, read back
    via the 2-byte xbar DMA transpose on a uint16 view; each u16
    partition holds a (kv, kv+1) byte pair = DoubleRow's [K,2,N] moving
    layout after bitcast+rearrange. v is stored pair-major to match.
  - score matmuls / exp / adds truncated to the 128-aligned causal
    boundary; attn tail memset to zero.
  - two half-size AllToAlls (one per batch) so the batch-0 output
    projection + LayerNorm overlaps batch-1 attention; residual
    pre-scaled x512 on host (LayerNorm is scale-invariant).

Scheduling (what got this from 471us to ~380us measured, ~330us
net of inter-core launch skew):
  - ALL bulk input loads ride ONE hwdge queue (sync) in consumption
    order with fat full-row descriptors (weights host-prearranged to
    [p,kc,f]); the 16 DMA engines fair-share across queues with pending
    work, so a single ordered queue completes the first kc-slabs in ~2us
    (first matmul at ~14us vs ~50us) while the rest stream underneath.
  - raw activations stay resident in SBUF (pin pool, released after the
    projections; Wo/gamma/beta/qres load into the freed space mid-flight).
  - one 8-bank PSUM pool for the whole kernel (ps:1 + ps_sc:3 + ps_ac:3
    + psv/cx:1) so projections overlap attention score matmuls; shifted
    bd is accumulated onto the ac PSUM by an fp8 identity matmul and exp
    reads the PSUM bank per 512-col chunk (partial row sums recombined
    on DVE), keeping DVE out of the score->softmax pacing chain.
  - ONE xbar transpose per attn head-region (out [p,w,q]: the extra dim
    extends the partition dim, so source col 128w+p lands at partition
    p, plane w) replaces 56 per-window transposes.
  - output projections emitted under tile_wait_until so the scheduler
    cannot hoist Wo in front of ready attention work (its cost model
    thinks the AllToAll is fast; on hw the collective absorbs 30-130us
    of inter-core launch skew and would head-of-line block the PE).
  - queue split: scalar hwdge = bd shifted reads + odd attn transposes;
    sync = inputs, recip, a2a, even transposes, out; gpsimd swdge =
    latency-tolerant bd raw writes + attn exports + late constants.
  - wide-causal q-half (hf=1) runs first per batch so the drain tail is
    the cheap half; LayerNorm rsqrt via Ln+Exp (same activation table as
    the attention Exp - Sqrt would force 1.3us table reloads).
"""

import numpy as np

# problem shapes (hardcoded per contract)
B, TQ, TKV, D, NH, DV = 2, 1024, 2048, 1024, 16, 64
N_CORES = 8
HPC = NH // N_CORES          # heads per core = 2
FPC = HPC * DV               # head-feature columns per core = 128
RPC = (B * TQ) // N_CORES    # output token rows per core = 256
R_OFF = TKV - TQ             # causal memory offset = 1024
LN_EPS = 1e-5
NT = TQ // 128               # query row chunks = 8
NK = TKV // 512              # key col chunks of 512 = 4
NW = TKV // 256              # kv windows of 256 per batch = 8

WS = 32.0                    # host weight pre-scale (fp8 range use)
CTX_S = 16.0                 # ctx fp8 pre-scale (folded into recip)
RES_S = WS * CTX_S           # residual pre-scale = 512

_CACHE = {}


def _patched_tc_class():
    """TileContext whose kernel-tail drain splits sem waits one per drain.

    The walrus build in this container rejects CTRL-type instructions
    (InstDrain) carrying more than one sync-wait command.
    """
    import concourse.mybir as mybir
    import concourse.tile as tile
    from concourse.vector_clock import ScopedClock

    class TC(tile.TileContext):
        def _commit_instruction(self, inst, lazy_reg_writes=True):
            # This walrus build rejects instructions carrying more than one
            # sync-wait command; hoist extras onto preceding NoOp carriers.
            si = getattr(inst, "sync_info", None)
            if (
                si is not None
                and si.on_wait
                and len(si.on_wait) > 1
                and inst.engine != mybir.EngineType.Unassigned
            ):
                waits = list(si.on_wait)
                inst.sync_info = mybir.SyncInfo(
                    on_wait=[waits[-1]], on_update=list(si.on_update or [])
                )
                for w in waits[:-1]:
                    ev = mybir.InstNoOp(
                        name=f"I-wsplit-{self.nc.next_id()}", ins=[], outs=[]
                    )
                    ev.engine = inst.engine
                    ev.sync_info = mybir.SyncInfo(on_wait=[w], on_update=[])
                    self._add_instruction(ev)
            return super()._commit_instruction(inst, lazy_reg_writes)

        def _drain_and_barrier(self, tick_clock, wait_clock):
            nc = self.nc
            drain_inst = nc.sync.drain()
            wait_clock.add_sem_waits(
                drain_inst.ins, ScopedClock({None: tick_clock.global_clock})
            )
            inner = drain_inst.ins
            si = inner.sync_info
            waits = list(si.on_wait) if si and si.on_wait else []
            if len(waits) > 1:
                inner.sync_info = mybir.SyncInfo(
                    on_wait=waits[:1], on_update=list(si.on_update or [])
                )
                for w in waits[1:]:
                    d2 = nc.sync.drain()
                    d2.ins.sync_info = mybir.SyncInfo(on_wait=[w], on_update=[])
            nc.all_engine_barrier()
            assert self.sems is not None
            popped = nc._tile_sem_poison_stack.pop()
            assert popped is self._sem_poison
            nc.clear_and_free_semaphores(list(self.sems.allocated().values()))
            nc.all_engine_barrier()

    return TC


def build_program(score_dtype="bfloat16", proj_dtype="float8e4", n_cores=N_CORES):
    """Build the SPMD Bass program (identical on all 8 cores).

    n_cores=1 builds a single-core variant (collectives replaced by DRAM
    copies) for profiling; its output is only valid for core 0's shard.
    """
    import concourse.bass as bass
    import concourse.mybir as mybir
    from concourse.bass import AP

    f32 = mybir.dt.float32
    bf16 = mybir.dt.bfloat16
    fp8 = mybir.dt.float8e4
    u16 = mybir.dt.uint16
    DR = mybir.MatmulPerfMode.DoubleRow

    nc = bass.Bass()

    # ---- I/O ----
    xqT = nc.dram_tensor("xqT", [D, B * TQ], fp8, kind="ExternalInput")
    xkvT = nc.dram_tensor("xkvT", [D, B * TKV], fp8, kind="ExternalInput")
    xrT = nc.dram_tensor("xrT", [D, B * TKV], fp8, kind="ExternalInput")
    # weights arrive host-prearranged as [p, kc, f] so each DMA descriptor
    # is a full per-partition row (1-8KB), not a 128B sliver
    wq = nc.dram_tensor("wq", [128, D // 128, FPC], fp8, kind="ExternalInput")
    wk = nc.dram_tensor("wk", [128, D // 128, FPC], fp8, kind="ExternalInput")
    wv = nc.dram_tensor("wv", [128, D // 128, FPC], fp8, kind="ExternalInput")
    wr = nc.dram_tensor("wr", [128, D // 128, FPC], fp8, kind="ExternalInput")
    wo = nc.dram_tensor("wo", [128, D // 128, D], fp8, kind="ExternalInput")
    cbv = nc.dram_tensor("cbv", [FPC, 1], f32, kind="ExternalInput")
    pbv = nc.dram_tensor("pbv", [FPC, 1], f32, kind="ExternalInput")
    qres = nc.dram_tensor("qres", [RPC, D], f32, kind="ExternalInput")
    gamma = nc.dram_tensor("gamma", [D], f32, kind="ExternalInput")
    beta = nc.dram_tensor("beta", [D], f32, kind="ExternalInput")
    out = nc.dram_tensor("out", [RPC, D], f32, kind="ExternalOutput")

    # ---- internal DRAM scratch ----
    # per-batch super-pair (both heads), two head regions per tensor;
    # bd split into q<512 / q>=512 halves so pass 2 starts early
    bd_dram = [
        [
            nc.dram_tensor(f"bd_dram{p}_{hf}", [2 * (TQ // 2) * TKV], bf16)
            for hf in range(2)
        ]
        for p in range(2)
    ]
    attn_dram = [
        [nc.dram_tensor(f"attn_dram{p}_{hf}", [TQ, TKV // 2], u16) for hf in range(2)]
        for p in range(2)
    ]
    recip_dram = [
        [nc.dram_tensor(f"recip_dram{p}_{hf}", [TQ], f32) for hf in range(2)]
        for p in range(2)
    ]
    a2a_in = [nc.dram_tensor(f"a2a_in{bc}", [N_CORES * FPC, TQ // 8], fp8) for bc in range(2)]
    a2a_out = [nc.dram_tensor(f"a2a_out{bc}", [N_CORES * FPC, TQ // 8], fp8) for bc in range(2)]

    Exp = mybir.ActivationFunctionType.Exp
    Identity = mybir.ActivationFunctionType.Identity
    Ln = mybir.ActivationFunctionType.Ln
    ALU = mybir.AluOpType
    AXL = mybir.AxisListType
    TC = _patched_tc_class()

    with TC(nc) as tc:
        import contextlib

        with contextlib.ExitStack() as ctx:
            singles = ctx.enter_context(tc.tile_pool(name="singles", bufs=1))

            # ---- static SBUF tensors ----
            wq_sb = singles.tile([128, D // 128, FPC], fp8, tag="wq_sb")
            wk_sb = singles.tile([128, D // 128, FPC], fp8, tag="wk_sb")
            wv_sb = singles.tile([128, D // 128, FPC], fp8, tag="wv_sb")
            wr_sb = singles.tile([128, D // 128, FPC], fp8, tag="wr_sb")
            nc.sync.dma_start(out=wq_sb, in_=wq[:])
            cb_sb = singles.tile([FPC, 1], f32, tag="cb_sb")
            pb_sb = singles.tile([FPC, 1], f32, tag="pb_sb")
            nc.sync.dma_start(out=cb_sb, in_=cbv[:])
            nc.sync.dma_start(out=pb_sb, in_=pbv[:])
            eps_sb = singles.tile([128, 1], f32, tag="eps_sb")
            nc.vector.memset(eps_sb, LN_EPS)

            # projection outputs (feature-major, both heads stacked on partitions)
            qcb_sb = singles.tile([FPC, B * TQ, 2], fp8, tag="qcb_sb")
            qpb_sb = singles.tile([FPC, B * TQ, 2], fp8, tag="qpb_sb")
            nc.vector.memset(qcb_sb[64:128, :, 0], 0.0)
            nc.vector.memset(qcb_sb[0:64, :, 1], 0.0)
            nc.vector.memset(qpb_sb[64:128, :, 0], 0.0)
            nc.vector.memset(qpb_sb[0:64, :, 1], 0.0)
            kT_sb = singles.tile([FPC, B * TKV], fp8, tag="kT_sb")
            rT_sb = singles.tile([FPC, B * TKV], fp8, tag="rT_sb")
            # v pair-major: [pair, window, parity, feat]
            v_pm = singles.tile([128, B * NW, 2, FPC], fp8, tag="v_pm")
            ctx_sb = singles.tile([FPC, B * TQ], fp8, tag="ctx_sb")

            ident_bf = singles.tile([128, 128], bf16, tag="ident_bf")
            ident_f8 = singles.tile([128, 128], fp8, tag="ident_f8")
            from concourse.masks import make_identity

            make_identity(nc, ident_bf)
            make_identity(nc, ident_f8)

            CH = 512  # token columns per projection step
            with contextlib.ExitStack() as pctx:
                pb_rows = pctx.enter_context(tc.tile_pool(name="pb_rows", bufs=3))
                pb_sum = pctx.enter_context(tc.tile_pool(name="pb_sum", bufs=3))
                pb_t = pctx.enter_context(tc.tile_pool(name="pb_t", bufs=4))
                pb_small = pctx.enter_context(tc.tile_pool(name="pb_small", bufs=6))
                pb_recip = pctx.enter_context(tc.tile_pool(name="pb_recip", bufs=2))
                pc = pctx.enter_context(tc.tile_pool(name="pc", bufs=3))
                pc_small = pctx.enter_context(tc.tile_pool(name="pc_small", bufs=8))
                fill_reg = nc.gpsimd.to_reg(-240.0)
                # single PSUM pool for the whole kernel so projections can
                # overlap attention: ps(2) + ps_sc(2) + ps_ac(2) + psv/cx(2)
                # = exactly 8 banks
                pps = tc.alloc_tile_pool(name="pps", bufs=2, space="PSUM")
                pa_vt = tc.alloc_tile_pool(name="pa_vt", bufs=2)
                # raw activations resident in SBUF, loaded with full-row
                # (2KB) descriptors in 5 big DMAs on the otherwise-idle DVE
                # queue, batch-0 halves first so attention starts early
                pin = tc.alloc_tile_pool(name="pin", bufs=1)
                xq_sb = pin.tile([128, D // 128, B * TQ], fp8, tag="xq")
                xkv_h = [
                    pin.tile([128, D // 128, TKV], fp8, tag=f"xkv{b}",
                             name=f"xkv_sb{b}")
                    for b in range(2)
                ]
                xr_h = [
                    pin.tile([128, D // 128, TKV], fp8, tag=f"xr{b}",
                             name=f"xr_sb{b}")
                    for b in range(2)
                ]
                # ALL bulk input loads go on the ONE sync HWDGE queue, in
                # consumption order: the 16 DMA engines fair-share across
                # queues with pending descriptors, so spreading the loads
                # over several queues makes everything finish together (~60us)
                # while a single ordered queue completes the first slabs in
                # ~2us and compute starts immediately. kc-slab granularity
                # keeps per-slab completion sems fine-grained.
                for k2 in range(4):
                    nc.sync.dma_start(
                        out=xq_sb[:, 2 * k2 : 2 * k2 + 2, :],
                        in_=xqT[:].rearrange("(kc p) t -> p kc t", p=128)[
                            :, 2 * k2 : 2 * k2 + 2, :
                        ],
                    )
                nc.sync.dma_start(out=wk_sb, in_=wk[:])
                nc.sync.dma_start(out=wv_sb, in_=wv[:])
                nc.sync.dma_start(out=wr_sb, in_=wr[:])
                for b_ in range(2):
                    for k4 in range(2):
                        nc.sync.dma_start(
                            out=xkv_h[b_][:, 4 * k4 : 4 * k4 + 4, :],
                            in_=xkvT[:].rearrange("(kc p) t -> p kc t", p=128)[
                                :, 4 * k4 : 4 * k4 + 4, b_ * TKV : (b_ + 1) * TKV
                            ],
                        )
                        nc.sync.dma_start(
                            out=xr_h[b_][:, 4 * k4 : 4 * k4 + 4, :],
                            in_=xrT[:].rearrange("(kc p) t -> p kc t", p=128)[
                                :, 4 * k4 : 4 * k4 + 4, b_ * TKV : (b_ + 1) * TKV
                            ],
                        )

                def proj_mm(ps, w_sb, x_in):
                    for kc2 in range(D // 256):
                        nc.tensor.matmul(
                            ps,
                            w_sb[:, 2 * kc2 : 2 * kc2 + 2, :],
                            x_in[:, 2 * kc2 : 2 * kc2 + 2, :],
                            start=(kc2 == 0),
                            stop=(kc2 == D // 256 - 1),
                            perf_mode=DR,
                        )

                def emit_q_chunk(j):
                    ps = pps.tile([FPC, CH], f32, tag="ps", bufs=1, name=f"ps_q{j}")
                    proj_mm(ps, wq_sb, xq_sb[:, :, j * CH : (j + 1) * CH])
                    sl = slice(j * CH, (j + 1) * CH)
                    for h_ in range(2):
                        hs = slice(64 * h_, 64 * h_ + 64)
                        nc.vector.tensor_scalar(
                            out=qcb_sb[hs, sl, h_], in0=ps[hs, :],
                            scalar1=1.0 / WS, scalar2=cb_sb[hs, :],
                            op0=ALU.mult, op1=ALU.add,
                        )
                        nc.vector.tensor_scalar(
                            out=qpb_sb[hs, sl, h_], in0=ps[hs, :],
                            scalar1=1.0 / WS, scalar2=pb_sb[hs, :],
                            op0=ALU.mult, op1=ALU.add,
                        )

                def emit_kvr_chunk(j):
                    b_, jj = j // 4, j % 4
                    kv_in = xkv_h[b_][:, :, jj * CH : (jj + 1) * CH]
                    sl = slice(j * CH, (j + 1) * CH)
                    ps = pps.tile([FPC, CH], f32, tag="ps", bufs=1, name=f"ps_k{j}")
                    proj_mm(ps, wk_sb, kv_in)
                    nc.vector.tensor_scalar_mul(
                        out=kT_sb[:, sl], in0=ps, scalar1=1.0 / WS
                    )
                    # v: feature-major, de-scale to bf16, then pair-major
                    # PE transposes into fp8 v_pm
                    psvt = pps.tile([FPC, CH], f32, tag="ps", bufs=1, name=f"psvt{j}")
                    proj_mm(psvt, wv_sb, kv_in)
                    vt_t = pa_vt.tile([FPC, CH], bf16, tag="vt_t", name=f"vt_t{j}")
                    nc.vector.tensor_scalar_mul(out=vt_t, in0=psvt, scalar1=1.0 / WS)
                    for w2 in range(CH // 256):
                        for par in range(2):
                            psv = pps.tile(
                                [128, FPC], bf16, tag="psv", bufs=1,
                                name=f"psv{j}_{w2}_{par}"
                            )
                            nc.tensor.transpose(
                                psv,
                                vt_t[:, 256 * w2 + par : 256 * (w2 + 1) : 2],
                                ident_bf,
                            )
                            if par == 0:
                                nc.scalar.copy(
                                    out=v_pm[:, j * 2 + w2, par, :], in_=psv
                                )
                            else:
                                nc.vector.tensor_copy(
                                    out=v_pm[:, j * 2 + w2, par, :], in_=psv
                                )
                    ps2 = pps.tile([FPC, CH], f32, tag="ps", bufs=1, name=f"ps_r{j}")
                    proj_mm(ps2, wr_sb, xr_h[b_][:, :, jj * CH : (jj + 1) * CH])
                    nc.vector.tensor_scalar_mul(
                        out=rT_sb[:, sl], in0=ps2, scalar1=1.0 / WS
                    )

                def emit_pass1(sp, hf):
                    # dual-tiles t2 in this q-half: 64 q-rows per head,
                    # both heads computed by one K=128 block-diag matmul
                    b = sp
                    for tl2 in range(8):
                        t2 = 8 * hf + tl2
                        c0 = max(0, (960 - 64 * t2) // 128 * 128)
                        bd_row = pb_rows.tile([128, TKV], bf16, tag="bd_row")
                        col = c0
                        while col < TKV:
                            cw = min(512 - col % 512, TKV - col)
                            ps_bd = pps.tile([128, 512], f32, tag="ps_sc", bufs=3)
                            nc.tensor.matmul(
                                ps_bd[:, :cw],
                                qpb_sb[:, b * TQ + 64 * t2 : b * TQ + 64 * t2 + 64, :],
                                rT_sb[:, b * TKV + col : b * TKV + col + cw],
                                start=True,
                                stop=True,
                            )
                            nc.vector.tensor_copy(
                                out=bd_row[:, col : col + cw], in_=ps_bd[:, :cw]
                            )
                            col += cw
                        # bd raw writes are latency-tolerant (consumed a full
                        # hf-phase later): SWDGE queue, keeping the scalar
                        # HWDGE free for the latency-critical shifted reads
                        # and attn transposes
                        nc.gpsimd.dma_start(
                            out=AP(
                                tensor=bd_dram[sp][hf][:].tensor,
                                offset=tl2 * 64 * TKV + c0,
                                ap=[[TKV, 64], [512 * TKV, 2], [1, TKV - c0]],
                            ),
                            in_=bd_row[:, c0:TKV],
                        )

                def emit_pass2(sp, hf):
                    b = sp
                    rmat = pb_recip.tile(
                        [128, 8], f32, tag="rmat", name=f"rm{sp}_{hf}"
                    )
                    for tl2 in range(8):
                        t2 = 8 * hf + tl2
                        ncc = 1152 + 128 * (t2 // 2)  # causal cols, 128-aligned
                        nc512 = 1536 if t2 < 8 else TKV
                        bd_shift = pb_rows.tile([128, TKV], bf16, tag="bd_shift")
                        nc.sync.dma_start(
                            out=bd_shift[:, :ncc],
                            in_=AP(
                                tensor=bd_dram[sp][hf][:].tensor,
                                offset=tl2 * 64 * TKV
                                + (TQ - 1)
                                - 512 * hf
                                - 64 * tl2,
                                ap=[[TKV - 1, 64], [512 * TKV, 2], [1, ncc]],
                            ),
                        )
                        # select iff 2*base0 + p - 2*j >= 0, i.e.
                        # j <= base0 + p//2 (p = 2c+h, both heads share c)
                        nc.gpsimd.affine_select(
                            out=bd_shift[:, ncc - 128 : ncc],
                            in_=bd_shift[:, ncc - 128 : ncc],
                            pattern=[[-2, 128]],
                            compare_op=ALU.is_ge,
                            fill=fill_reg,
                            base=128 * (t2 % 2),
                            channel_multiplier=1,
                        )
                        attn_row = pb_sum.tile([128, TKV], fp8, tag="attn_row")
                        if ncc < nc512:
                            nc.gpsimd.memset(attn_row[:, ncc:nc512], 0.0)
                        # ac+bd summed ON the PE: the ac matmul opens the PSUM
                        # group and an fp8 identity matmul accumulates the
                        # shifted bd on top; exp then reads the PSUM bank
                        # directly (per chunk, partial row sums combined on
                        # DVE). This removes the DVE tensor_add from the
                        # ac->exp pacing chain entirely.
                        nch = (ncc + 511) // 512
                        rs4 = pb_small.tile([128, 4], f32, tag="rs4")
                        for n in range(nch):
                            cw = min(512, ncc - 512 * n)
                            ps_ac = pps.tile([128, 512], f32, tag="ps_ac", bufs=3)
                            nc.tensor.matmul(
                                ps_ac[:, :cw],
                                qcb_sb[:, b * TQ + 64 * t2 : b * TQ + 64 * t2 + 64, :],
                                kT_sb[:, b * TKV + 512 * n : b * TKV + 512 * n + cw],
                                start=True,
                                stop=False,
                            )
                            nc.tensor.matmul(
                                ps_ac[:, :cw],
                                ident_bf,
                                bd_shift[:, 512 * n : 512 * n + cw],
                                start=False,
                                stop=True,
                            )
                            nc.scalar.activation(
                                out=attn_row[:, 512 * n : 512 * n + cw],
                                in_=ps_ac[:, :cw],
                                func=Exp,
                                scale=0.125,
                                accum_out=rs4[:, n : n + 1],
                            )
                        rowsum = pb_small.tile([128, 1], f32, tag="rowsum")
                        nc.vector.tensor_reduce(
                            out=rowsum, in_=rs4[:, :nch], axis=AXL.X, op=ALU.add
                        )
                        recip = pb_small.tile([128, 1], f32, tag="recip")
                        nc.vector.reciprocal(recip, rowsum)
                        nc.vector.tensor_scalar_mul(
                            out=rmat[:, tl2 : tl2 + 1], in0=recip, scalar1=CTX_S
                        )
                        nc.gpsimd.dma_start(
                            out=AP(
                                tensor=attn_dram[sp][hf][:].tensor,
                                offset=64 * tl2 * (TKV // 2),
                                ap=[
                                    [TKV // 2, 64],
                                    [512 * (TKV // 2), 2],
                                    [1, nc512 // 2],
                                ],
                            ),
                            in_=attn_row[:, :nc512].bitcast(u16),
                        )
                    # export recip: flat interleaved layout (2*q_local + head)
                    nc.sync.dma_start(
                        out=AP(
                            tensor=recip_dram[sp][hf][:].tensor,
                            offset=0,
                            ap=[[1, 128], [128, 8]],
                        ),
                        in_=rmat,
                    )

                ctx_ps = {}

                def emit_ctx_nn(sp, hh, n_):
                    b = sp
                    qf = slice(64 * hh, 64 * hh + 64)
                    rb = pb_t.tile(
                        [64, 1024], f32, tag="recip_bc", name=f"rb{sp}_{hh}_{n_}"
                    )
                    nc.sync.dma_start(
                        out=rb,
                        in_=AP(
                            tensor=recip_dram[sp][n_][:].tensor,
                            offset=0,
                            ap=[[0, 64], [1, 1024]],
                        ),
                    )
                    ps_cx = pps.tile(
                        [64, 512], f32, tag="psv", bufs=1,
                        name=f"ps_cx{sp}_{hh}_{n_}"
                    )
                    nws = 6 if n_ == 0 else NW
                    # ONE region transpose instead of one per 128-col window:
                    # xbar out [p, w, q] extends the partition dim with the
                    # extra dim, so source col 128w+p lands at partition p,
                    # plane w - exactly the per-window layout, 8 instructions
                    # instead of 56 on the hwdge queues
                    atr = pb_t.tile(
                        [128, NW, 512], u16, tag="attnT2b", bufs=2,
                        name=f"atr{sp}_{hh}_{n_}"
                    )
                    eng = nc.sync if hh == 0 else nc.scalar
                    eng.dma_start(
                        out=atr[:, :nws, :],
                        in_=attn_dram[sp][n_][
                            512 * hh : 512 * hh + 512, 0 : 128 * nws
                        ],
                        transpose=True,
                    )
                    for w in range(nws):
                        rr = (
                            atr[:, w, :]
                            .bitcast(fp8)
                            .rearrange("p (q i) -> p i q", i=2)
                        )
                        nc.tensor.matmul(
                            ps_cx,
                            v_pm[:, b * NW + w, :, qf],
                            rr,
                            start=(w == 0),
                            stop=(w == nws - 1),
                            perf_mode=DR,
                        )
                    nc.vector.tensor_mul(
                        out=ctx_sb[qf, b * TQ + 512 * n_ : b * TQ + 512 * (n_ + 1)],
                        in0=ps_cx,
                        in1=rb[:, hh::2],
                    )

                def emit_a2a(bc):
                    nc.sync.dma_start(
                        out=a2a_in[bc][:].rearrange("(j p) t -> p j t", p=FPC),
                        in_=ctx_sb[:, bc * TQ : (bc + 1) * TQ].rearrange(
                            "p (j t) -> p j t", t=TQ // 8
                        ),
                    )
                    if n_cores > 1:
                        nc.gpsimd.collective_compute(
                            "AllToAll",
                            ALU.bypass,
                            replica_groups=[list(range(n_cores))],
                            ins=[a2a_in[bc][:]],
                            outs=[a2a_out[bc][:]],
                        )
                    else:
                        nc.sync.dma_start(out=a2a_out[bc][:], in_=a2a_in[bc][:])

                def emit_phase_c(bc):
                    ps_o = [
                        pps.tile([128, 512], f32, tag="ps_sc", bufs=3, name=f"ps_o{bc}_{nn_}")
                        for nn_ in range(2)
                    ]
                    for kc2 in range(D // 256):
                        lhs = pc.tile([128, 2, 128], fp8, tag="octx")
                        nc.sync.dma_start(
                            out=lhs,
                            in_=a2a_out[bc][
                                kc2 * 256 : (kc2 + 1) * 256, :
                            ].rearrange("(i p) t -> p i t", p=128),
                        )
                        for nn in range(2):
                            nc.tensor.matmul(
                                ps_o[nn],
                                lhs,
                                wo_sb[:, 2 * kc2 : 2 * kc2 + 2, nn * 512 : (nn + 1) * 512],
                                start=(kc2 == 0),
                                stop=(kc2 == D // 256 - 1),
                                perf_mode=DR,
                            )
                    o_sb = pc.tile([128, D], f32, tag="o_sb")
                    for nn in range(2):
                        nc.vector.tensor_add(
                            out=o_sb[:, nn * 512 : (nn + 1) * 512],
                            in0=ps_o[nn],
                            in1=qres_sb[:, bc, nn * 512 : (nn + 1) * 512],
                        )
                    # LayerNorm over the free (feature) dim
                    stats = pc_small.tile([128, 2, 6], f32, tag="stats")
                    for sg in range(2):
                        nc.vector.bn_stats(
                            out=stats[:, sg, :], in_=o_sb[:, sg * 512 : (sg + 1) * 512]
                        )
                    mv = pc_small.tile([128, 2], f32, tag="mv")
                    nc.vector.bn_aggr(out=mv, in_=stats)
                    mean, var = mv[:, 0:1], mv[:, 1:2]
                    xve = pc_small.tile([128, 1], f32, tag="xve")
                    nc.vector.tensor_scalar_add(out=xve, in0=var, scalar1=eps_sb)
                    # rsqrt = exp(-0.5*ln(var+eps)): Ln+Exp live in the same
                    # activation table as the attention Exp (Sqrt does not,
                    # and would force a 1.3us table reload each LayerNorm)
                    lnv = pc_small.tile([128, 1], f32, tag="lnv")
                    nc.scalar.activation(out=lnv, in_=var, func=Ln, bias=eps_sb)
                    rstd = pc_small.tile([128, 1], f32, tag="rstd")
                    nc.scalar.activation(out=rstd, in_=lnv, func=Exp, scale=-0.5)
                    # one Newton step for rsqrt accuracy:
                    # r <- r * (1.5 - 0.5 * x * r^2)
                    tnw = pc_small.tile([128, 1], f32, tag="tnw")
                    nc.vector.tensor_mul(out=tnw, in0=rstd, in1=rstd)
                    nc.vector.tensor_mul(out=tnw, in0=tnw, in1=xve)
                    nc.vector.tensor_scalar(
                        out=tnw, in0=tnw, scalar1=-0.5, scalar2=1.5,
                        op0=ALU.mult, op1=ALU.add,
                    )
                    nc.vector.tensor_scalar_mul(out=rstd, in0=rstd, scalar1=tnw)
                    nc.vector.tensor_scalar(
                        out=o_sb, in0=o_sb, scalar1=mean, scalar2=rstd,
                        op0=ALU.subtract, op1=ALU.mult,
                    )
                    nc.vector.tensor_mul(out=o_sb, in0=o_sb, in1=gamma_sb)
                    nc.vector.tensor_add(out=o_sb, in0=o_sb, in1=beta_sb)
                    nc.sync.dma_start(
                        out=out[bc * 128 : (bc + 1) * 128, :], in_=o_sb
                    )

                for j in range(4):
                    emit_q_chunk(j)
                for j in range(8):
                    emit_kvr_chunk(j)
                pin.release()
                pa_vt.release()
                # output-phase constants load into the space freed by the raw
                # activations, during the attention phase
                late = tc.alloc_tile_pool(name="late", bufs=1)
                wo_sb = late.tile([128, D // 128, D], fp8, tag="wo_sb")
                nc.gpsimd.dma_start(out=wo_sb, in_=wo[:])
                gamma_sb = late.tile([128, D], f32, tag="gamma_sb")
                beta_sb = late.tile([128, D], f32, tag="beta_sb")
                nc.gpsimd.dma_start(
                    out=gamma_sb,
                    in_=AP(tensor=gamma[:].tensor, offset=0, ap=[[0, 128], [1, D]]),
                )
                nc.gpsimd.dma_start(
                    out=beta_sb,
                    in_=AP(tensor=beta[:].tensor, offset=0, ap=[[0, 128], [1, D]]),
                )
                qres_sb = late.tile([128, RPC // 128, D], f32, tag="qres_sb")
                nc.gpsimd.dma_start(
                    out=qres_sb, in_=qres[:].rearrange("(mc p) d -> p mc d", p=128)
                )
                # zero-init bd scratch regions the shifted reads can touch but
                # raw writes never cover (wrap into the next row's low cols)
                zeros_f8 = pb_t.tile([128, 2048], fp8, tag="zeros_f8")
                nc.vector.memset(zeros_f8, 0.0)

                def zero_scratch(sp):
                    # pre-zero so any racy early read sees 0.0, never an fp8
                    # NaN byte (uninitialized DRAM): masked positions are
                    # replaced downstream, unmasked races stay tiny+finite
                    for hf in range(2):
                        for blk in range(8):
                            nc.sync.dma_start(
                                out=AP(
                                    tensor=bd_dram[sp][hf][:].tensor,
                                    offset=blk * 128 * TKV,
                                    ap=[[TKV, 128], [1, TKV]],
                                ),
                                in_=zeros_f8,
                            )
                    if sp == 0:
                        for bc in range(2):
                            nc.gpsimd.dma_start(
                                out=a2a_out[bc][:].rearrange(
                                    "(j p) t -> p j t", p=FPC
                                ),
                                in_=zeros_f8[:, : TQ // 8 * 8].rearrange(
                                    "p (j t) -> p j t", j=8
                                ),
                            )

                # interleaved emission: independent PE work between the
                # dependent stages. The AllToAll triggers (emit_a2a) are
                # decoupled from their consumers (emit_phase_c) so the PE's
                # in-order queue is never head-of-line blocked waiting on a
                # collective: both output projections run at the end, with
                # batch-0's overlapping batch-1's AllToAll latency.
                # hf=1 (the wide-causal half) runs FIRST within each batch so
                # the drain tail is the cheap half; each ctx pair follows its
                # producing pass2 immediately so attn-transpose chains overlap
                # the other half's score work
                # zero_scratch is NOT emitted: internal DRAM is
                # zero-initialized at NEFF load, and the 4MB of zero writes
                # were delaying the sync-queue attn transposes behind them
                emit_pass1(0, 1)
                emit_pass2(0, 1)
                emit_pass1(0, 0)
                emit_pass2(0, 0)
                emit_ctx_nn(0, 0, 1)
                emit_ctx_nn(0, 1, 1)
                emit_pass1(1, 1)
                emit_ctx_nn(0, 0, 0)
                emit_ctx_nn(0, 1, 0)
                emit_a2a(0)
                emit_pass2(1, 1)
                emit_pass1(1, 0)
                emit_pass2(1, 0)
                emit_ctx_nn(1, 0, 1)
                emit_ctx_nn(1, 1, 1)
                emit_ctx_nn(1, 0, 0)
                emit_ctx_nn(1, 1, 0)
                emit_a2a(1)
                # schedule the output projections LAST: the tile scheduler's
                # cost model thinks the AllToAll is fast and would otherwise
                # hoist the Wo matmuls in front of ready attention work,
                # head-of-line blocking the PE on the collective for ~35us
                with tc.tile_wait_until(1.0):
                    emit_phase_c(0)
                with tc.tile_wait_until(1.1):
                    emit_phase_c(1)
                late.release()
                pps.release()
    return nc


def _make_in_maps(inputs, mm_dtype="float8e4"):
    import ml_dtypes

    f8 = ml_dtypes.float8_e4m3

    query = np.asarray(inputs["query"], np.float32)
    key_value = np.asarray(inputs["key_value"], np.float32)
    relative = np.asarray(inputs["relative"], np.float32)
    content_bias = np.asarray(inputs["content_bias"], np.float32)
    position_bias = np.asarray(inputs["position_bias"], np.float32)
    Wq, Wk = np.asarray(inputs["Wq"], np.float32), np.asarray(inputs["Wk"], np.float32)
    Wv, Wr = np.asarray(inputs["Wv"], np.float32), np.asarray(inputs["Wr"], np.float32)
    Wo = np.ascontiguousarray(np.asarray(inputs["Wo"], np.float32))
    ln_gamma = np.asarray(inputs["ln_gamma"], np.float32)
    ln_beta = np.asarray(inputs["ln_beta"], np.float32)

    qflat = query.reshape(B * TQ, D)
    xqT = np.ascontiguousarray(qflat.T).astype(f8)
    xkvT = np.ascontiguousarray(key_value.reshape(B * TKV, D).T).astype(f8)
    xrT = np.ascontiguousarray(relative.reshape(B * TKV, D).T).astype(f8)
    Wq8 = (Wq * WS).astype(f8)
    Wk8 = (Wk * WS).astype(f8)
    Wv8 = (Wv * WS).astype(f8)
    Wr8 = (Wr * WS).astype(f8)
    Wo8 = (Wo * WS).astype(f8)
    cb = content_bias.reshape(NH, DV)
    pb = position_bias.reshape(NH, DV)

    def parr(w):  # [D, F] -> [p, kc, F]: per-partition rows for fat descriptors
        return np.ascontiguousarray(
            w.reshape(D // 128, 128, w.shape[1]).transpose(1, 0, 2)
        )

    in_maps = []
    for c in range(N_CORES):
        fs = slice(FPC * c, FPC * (c + 1))
        # out rows: [0,128) = batch-0 tokens [128c,+128),
        #           [128,256) = batch-1 tokens [128c,+128)
        qr = np.concatenate(
            [
                qflat[128 * c : 128 * c + 128],
                qflat[TQ + 128 * c : TQ + 128 * c + 128],
            ]
        )
        in_maps.append(
            {
                "xqT": xqT,
                "xkvT": xkvT,
                "xrT": xrT,
                "wq": parr(Wq8[:, fs]),
                "wk": parr(Wk8[:, fs]),
                "wv": parr(Wv8[:, fs]),
                "wr": parr(Wr8[:, fs]),
                "wo": parr(Wo8),
                "cbv": np.ascontiguousarray(
                    cb[HPC * c : HPC * (c + 1)].reshape(FPC, 1)
                ),
                "pbv": np.ascontiguousarray(
                    pb[HPC * c : HPC * (c + 1)].reshape(FPC, 1)
                ),
                "qres": np.ascontiguousarray(qr) * RES_S,
                "gamma": ln_gamma,
                "beta": ln_beta,
            }
        )
    return in_maps


def run_on_hw(inputs, trace=False, score_dtype="bfloat16", proj_dtype="float8e4"):
    from concourse.bass_utils import run_bass_kernel_spmd

    key = ("v3",)
    nc = _CACHE.get(key)
    if nc is None:
        nc = build_program()
        _CACHE[key] = nc
    in_maps = _make_in_maps(inputs)
    res = run_bass_kernel_spmd(nc, in_maps, list(range(N_CORES)), trace=trace)
    # core c rows: [b*128, b*128+128) = batch b tokens [128c, 128c+128)
    outs = np.stack(
        [np.asarray(res.results[c]["out"]) for c in range(N_CORES)]
    )  # [8, 256, D]
    outs = outs.reshape(N_CORES, B, 128, D).transpose(1, 0, 2, 3)
    return np.ascontiguousarray(outs.reshape(B, TQ, D)), res


def kernel(**inputs) -> np.ndarray:
    out, _ = run_on_hw(inputs)
    return out



# revision 50
# speedup vs baseline: 1.3409x; 1.3409x over previous
"""Trainium2 Bass kernel for Transformer-XL style relative-position attention.

Reference computation (B=2, Tq=1024, Tkv=2048, D=1024, H=16, Dv=64):
    q/k/v/r projections, ac = (q+cb)@k^T, bd = rel_shift((q+pb)@r^T),
    softmax((ac+bd)/8) with causal-with-memory mask, ctx = attn@v,
    out = LN(ctx@Wo + query).

Sharding (Megatron-style tensor parallel over heads, 8 cores):
  - each core owns 2 heads: Wq/Wk/Wv/Wr column shards [1024,128], biases
    shard with heads; activations broadcast (feature-major on host).
  - ctx^T shards exchanged with TWO half-size AllToAlls (one per batch) so
    the batch-0 output projection + LayerNorm overlaps batch-1 attention.
  - each core ends up with full ctx^T for 128 tokens of each batch:
    out rows [0,128) = batch-0 tokens [128c,128c+128), rows [128,256) =
    batch-1 tokens [128c,128c+128).

Device-side numerics (|ctx@Wo| is ~1.4% of |query|, so the attention path
tolerates fp8):
  - projections / ctx / output projection in fp8e4 + DoubleRow perf mode
    (2 contraction planes per pass); weights pre-scaled x32 on host,
    de-scaled on the PSUM->SBUF copy.
  - scores run both heads in ONE K=128 fp8 matmul: the stationary holds a
    block-diagonal dual-head q tile (head h's features x its 64 q rows,
    zeros elsewhere) with the two heads interleaved in the M dim
    (partition 2c+h), because the PE streams moving data at HALF rate
    when the contraction uses <=64 partitions.
  - rel_shift via flat DRAM scratch (write rows, read back at offset
    1023-q with row stride Tkv-1), fp8, split per q-half and per head
    region so pass 2 of a batch starts when half of pass 1 has landed.
  - shifted bd added to the ac PSUM on DVE; causal mask via one
    affine_select with a stride -2 iota (j <= base + p//2); exp on
    ScalarE writes UNNORMALIZED fp8 attn (fp8 normal range; accum_out
    row sums are f32 exact); 1/rowsum (x16 ctx scale) folded into the
    ctx PSUM->SBUF copy via an interleaved reciprocal broadcast read.
  - attn round trip: fp8 written contiguously per head region, read back
    via the 2-byte xbar DMA transpose on a uint16 view; each u16
    partition holds a (kv, kv+1) byte pair = DoubleRow's [K,2,N] moving
    layout after bitcast+rearrange. v is stored pair-major to match.
  - score matmuls / exp / adds truncated to the 128-aligned causal
    boundary; attn tail memset to zero.
  - two half-size AllToAlls (one per batch) so the batch-0 output
    projection + LayerNorm overlaps batch-1 attention; residual
    pre-scaled x512 on host (LayerNorm is scale-invariant).

Scheduling (what got this from 471us to ~380us measured, ~330us
net of inter-core launch skew):
  - ALL bulk input loads ride ONE hwdge queue (sync) in consumption
    order with fat full-row descriptors (weights host-prearranged to
    [p,kc,f]); the 16 DMA engines fair-share across queues with pending
    work, so a single ordered queue completes the first kc-slabs in ~2us
    (first matmul at ~14us vs ~50us) while the rest stream underneath.
  - raw activations stay resident in SBUF (pin pool, released after the
    projections; Wo/gamma/beta/qres load into the freed space mid-flight).
  - one 8-bank PSUM pool for the whole kernel (ps:1 + ps_sc:3 + ps_ac:3
    + psv/cx:1) so projections overlap attention score matmuls; shifted
    bd is accumulated onto the ac PSUM by an fp8 identity matmul and exp
    reads the PSUM bank per 512-col chunk (partial row sums recombined
    on DVE), keeping DVE out of the score->softmax pacing chain.
  - ONE xbar transpose per attn head-region (out [p,w,q]: the extra dim
    extends the partition dim, so source col 128w+p lands at partition
    p, plane w) replaces 56 per-window transposes.
  - output projections emitted under tile_wait_until so the scheduler
    cannot hoist Wo in front of ready attention work (its cost model
    thinks the AllToAll is fast; on hw the collective absorbs 30-130us
    of inter-core launch skew and would head-of-line block the PE).
  - queue split: scalar hwdge = bd shifted reads + odd attn transposes;
    sync = inputs, recip, a2a, even transposes, out; gpsimd swdge =
    latency-tolerant bd raw writes + attn exports + late constants.
  - wide-causal q-half (hf=1) runs first per batch so the drain tail is
    the cheap half; LayerNorm rsqrt via Ln+Exp (same activation table as
    the attention Exp - Sqrt would force 1.3us table reloads).
"""

import numpy as np

# problem shapes (hardcoded per contract)
B, TQ, TKV, D, NH, DV = 2, 1024, 2048, 1024, 16, 64
N_CORES = 8
HPC = NH // N_CORES          # heads per core = 2
FPC = HPC * DV               # head-feature columns per core = 128
RPC = (B * TQ) // N_CORES    # output token rows per core = 256
R_OFF = TKV - TQ             # causal memory offset = 1024
LN_EPS = 1e-5
NT = TQ // 128               # query row chunks = 8
NK = TKV // 512              # key col chunks of 512 = 4
NW = TKV // 256              # kv windows of 256 per batch = 8

WS = 32.0                    # host weight pre-scale (fp8 range use)
CTX_S = 16.0                 # ctx fp8 pre-scale (folded into recip)
RES_S = WS * CTX_S           # residual pre-scale = 512

_CACHE = {}


def _patched_tc_class():
    """TileContext whose kernel-tail drain splits sem waits one per drain.

    The walrus build in this container rejects CTRL-type instructions
    (InstDrain) carrying more than one sync-wait command.
    """
    import concourse.mybir as mybir
    import concourse.tile as tile
    from concourse.vector_clock import ScopedClock

    class TC(tile.TileContext):
        def _commit_instruction(self, inst, lazy_reg_writes=True):
            # This walrus build rejects instructions carrying more than one
            # sync-wait command; hoist extras onto preceding NoOp carriers.
            si = getattr(inst, "sync_info", None)
            if (
                si is not None
                and si.on_wait
                and len(si.on_wait) > 1
                and inst.engine != mybir.EngineType.Unassigned
            ):
                waits = list(si.on_wait)
                inst.sync_info = mybir.SyncInfo(
                    on_wait=[waits[-1]], on_update=list(si.on_update or [])
                )
                for w in waits[:-1]:
                    ev = mybir.InstNoOp(
                        name=f"I-wsplit-{self.nc.next_id()}", ins=[], outs=[]
                    )
                    ev.engine = inst.engine
                    ev.sync_info = mybir.SyncInfo(on_wait=[w], on_update=[])
                    self._add_instruction(ev)
            return super()._commit_instruction(inst, lazy_reg_writes)

        def _drain_and_barrier(self, tick_clock, wait_clock):
            nc = self.nc
            drain_inst = nc.sync.drain()
            wait_clock.add_sem_waits(
                drain_inst.ins, ScopedClock({None: tick_clock.global_clock})
            )
            inner = drain_inst.ins
            si = inner.sync_info
            waits = list(si.on_wait) if si and si.on_wait else []
            if len(waits) > 1:
                inner.sync_info = mybir.SyncInfo(
                    on_wait=waits[:1], on_update=list(si.on_update or [])
                )
                for w in waits[1:]:
                    d2 = nc.sync.drain()
                    d2.ins.sync_info = mybir.SyncInfo(on_wait=[w], on_update=[])
            nc.all_engine_barrier()
            assert self.sems is not None
            popped = nc._tile_sem_poison_stack.pop()
            assert popped is self._sem_poison
            nc.clear_and_free_semaphores(list(self.sems.allocated().values()))
            nc.all_engine_barrier()

    return TC


def build_program(score_dtype="bfloat16", proj_dtype="float8e4", n_cores=N_CORES):
    """Build the SPMD Bass program (identical on all 8 cores).

    n_cores=1 builds a single-core variant (collectives replaced by DRAM
    copies) for profiling; its output is only valid for core 0's shard.
    """
    import concourse.bass as bass
    import concourse.mybir as mybir
    from concourse.bass import AP

    f32 = mybir.dt.float32
    bf16 = mybir.dt.bfloat16
    fp8 = mybir.dt.float8e4
    u16 = mybir.dt.uint16
    DR = mybir.MatmulPerfMode.DoubleRow

    nc = bass.Bass()

    # ---- I/O ----
    xqT = nc.dram_tensor("xqT", [D, B * TQ], fp8, kind="ExternalInput")
    xkvT = nc.dram_tensor("xkvT", [D, B * TKV], fp8, kind="ExternalInput")
    xrT = nc.dram_tensor("xrT", [D, B * TKV], fp8, kind="ExternalInput")
    # weights arrive host-prearranged as [p, kc, f] so each DMA descriptor
    # is a full per-partition row (1-8KB), not a 128B sliver
    wq = nc.dram_tensor("wq", [128, D // 128, FPC], fp8, kind="ExternalInput")
    wk = nc.dram_tensor("wk", [128, D // 128, FPC], fp8, kind="ExternalInput")
    wv = nc.dram_tensor("wv", [128, D // 128, FPC], fp8, kind="ExternalInput")
    wr = nc.dram_tensor("wr", [128, D // 128, FPC], fp8, kind="ExternalInput")
    wo = nc.dram_tensor("wo", [128, D // 128, D], fp8, kind="ExternalInput")
    cbv = nc.dram_tensor("cbv", [FPC, 1], f32, kind="ExternalInput")
    pbv = nc.dram_tensor("pbv", [FPC, 1], f32, kind="ExternalInput")
    qres = nc.dram_tensor("qres", [RPC, D], f32, kind="ExternalInput")
    gamma = nc.dram_tensor("gamma", [D], f32, kind="ExternalInput")
    beta = nc.dram_tensor("beta", [D], f32, kind="ExternalInput")
    out = nc.dram_tensor("out", [RPC, D], f32, kind="ExternalOutput")

    # ---- internal DRAM scratch ----
    # per-batch super-pair (both heads), two head regions per tensor;
    # bd split into q<512 / q>=512 halves so pass 2 starts early
    bd_dram = [
        [
            nc.dram_tensor(f"bd_dram{p}_{hf}", [2 * (TQ // 2) * TKV], fp8)
            for hf in range(2)
        ]
        for p in range(2)
    ]
    attn_dram = [
        [nc.dram_tensor(f"attn_dram{p}_{hf}", [TQ, TKV // 2], u16) for hf in range(2)]
        for p in range(2)
    ]
    recip_dram = [
        [nc.dram_tensor(f"recip_dram{p}_{hf}", [TQ], f32) for hf in range(2)]
        for p in range(2)
    ]
    a2a_in = [nc.dram_tensor(f"a2a_in{bc}", [N_CORES * FPC, TQ // 8], fp8) for bc in range(2)]
    a2a_out = [nc.dram_tensor(f"a2a_out{bc}", [N_CORES * FPC, TQ // 8], fp8) for bc in range(2)]

    Exp = mybir.ActivationFunctionType.Exp
    Identity = mybir.ActivationFunctionType.Identity
    Ln = mybir.ActivationFunctionType.Ln
    ALU = mybir.AluOpType
    AXL = mybir.AxisListType
    TC = _patched_tc_class()

    with TC(nc) as tc:
        import contextlib

        with contextlib.ExitStack() as ctx:
            singles = ctx.enter_context(tc.tile_pool(name="singles", bufs=1))

            # ---- static SBUF tensors ----
            wq_sb = singles.tile([128, D // 128, FPC], fp8, tag="wq_sb")
            wk_sb = singles.tile([128, D // 128, FPC], fp8, tag="wk_sb")
            wv_sb = singles.tile([128, D // 128, FPC], fp8, tag="wv_sb")
            wr_sb = singles.tile([128, D // 128, FPC], fp8, tag="wr_sb")
            nc.sync.dma_start(out=wq_sb, in_=wq[:])
            cb_sb = singles.tile([FPC, 1], f32, tag="cb_sb")
            pb_sb = singles.tile([FPC, 1], f32, tag="pb_sb")
            nc.sync.dma_start(out=cb_sb, in_=cbv[:])
            nc.sync.dma_start(out=pb_sb, in_=pbv[:])
            eps_sb = singles.tile([128, 1], f32, tag="eps_sb")
            nc.vector.memset(eps_sb, LN_EPS)

            # projection outputs (feature-major, both heads stacked on partitions)
            qcb_sb = singles.tile([FPC, B * TQ, 2], fp8, tag="qcb_sb")
            qpb_sb = singles.tile([FPC, B * TQ, 2], fp8, tag="qpb_sb")
            nc.vector.memset(qcb_sb[64:128, :, 0], 0.0)
            nc.vector.memset(qcb_sb[0:64, :, 1], 0.0)
            nc.vector.memset(qpb_sb[64:128, :, 0], 0.0)
            nc.vector.memset(qpb_sb[0:64, :, 1], 0.0)
            kT_sb = singles.tile([FPC, B * TKV], fp8, tag="kT_sb")
            rT_sb = singles.tile([FPC, B * TKV], fp8, tag="rT_sb")
            # v pair-major: [pair, window, parity, feat]
            v_pm = singles.tile([128, B * NW, 2, FPC], fp8, tag="v_pm")
            ctx_sb = singles.tile([FPC, B * TQ], fp8, tag="ctx_sb")

            ident_bf = singles.tile([128, 128], bf16, tag="ident_bf")
            ident_f8 = singles.tile([128, 128], fp8, tag="ident_f8")
            from concourse.masks import make_identity

            make_identity(nc, ident_bf)
            make_identity(nc, ident_f8)

            CH = 512  # token columns per projection step
            with contextlib.ExitStack() as pctx:
                pb_rows = pctx.enter_context(tc.tile_pool(name="pb_rows", bufs=3))
                pb_sum = pctx.enter_context(tc.tile_pool(name="pb_sum", bufs=3))
                pb_t = pctx.enter_context(tc.tile_pool(name="pb_t", bufs=4))
                pb_small = pctx.enter_context(tc.tile_pool(name="pb_small", bufs=6))
                pb_recip = pctx.enter_context(tc.tile_pool(name="pb_recip", bufs=2))
                pc = pctx.enter_context(tc.tile_pool(name="pc", bufs=3))
                pc_small = pctx.enter_context(tc.tile_pool(name="pc_small", bufs=8))
                fill_reg = nc.gpsimd.to_reg(-240.0)
                # single PSUM pool for the whole kernel so projections can
                # overlap attention: ps(2) + ps_sc(2) + ps_ac(2) + psv/cx(2)
                # = exactly 8 banks
                pps = tc.alloc_tile_pool(name="pps", bufs=2, space="PSUM")
                pa_vt = tc.alloc_tile_pool(name="pa_vt", bufs=2)
                # raw activations resident in SBUF, loaded with full-row
                # (2KB) descriptors in 5 big DMAs on the otherwise-idle DVE
                # queue, batch-0 halves first so attention starts early
                pin = tc.alloc_tile_pool(name="pin", bufs=1)
                xq_sb = pin.tile([128, D // 128, B * TQ], fp8, tag="xq")
                xkv_h = [
                    pin.tile([128, D // 128, TKV], fp8, tag=f"xkv{b}",
                             name=f"xkv_sb{b}")
                    for b in range(2)
                ]
                xr_h = [
                    pin.tile([128, D // 128, TKV], fp8, tag=f"xr{b}",
                             name=f"xr_sb{b}")
                    for b in range(2)
                ]
                # ALL bulk input loads go on the ONE sync HWDGE queue, in
                # consumption order: the 16 DMA engines fair-share across
                # queues with pending descriptors, so spreading the loads
                # over several queues makes everything finish together (~60us)
                # while a single ordered queue completes the first slabs in
                # ~2us and compute starts immediately. kc-slab granularity
                # keeps per-slab completion sems fine-grained.
                for k2 in range(4):
                    nc.sync.dma_start(
                        out=xq_sb[:, 2 * k2 : 2 * k2 + 2, :],
                        in_=xqT[:].rearrange("(kc p) t -> p kc t", p=128)[
                            :, 2 * k2 : 2 * k2 + 2, :
                        ],
                    )
                nc.sync.dma_start(out=wk_sb, in_=wk[:])
                nc.sync.dma_start(out=wv_sb, in_=wv[:])
                nc.sync.dma_start(out=wr_sb, in_=wr[:])
                for b_ in range(2):
                    for k4 in range(2):
                        nc.sync.dma_start(
                            out=xkv_h[b_][:, 4 * k4 : 4 * k4 + 4, :],
                            in_=xkvT[:].rearrange("(kc p) t -> p kc t", p=128)[
                                :, 4 * k4 : 4 * k4 + 4, b_ * TKV : (b_ + 1) * TKV
                            ],
                        )
                        nc.sync.dma_start(
                            out=xr_h[b_][:, 4 * k4 : 4 * k4 + 4, :],
                            in_=xrT[:].rearrange("(kc p) t -> p kc t", p=128)[
                                :, 4 * k4 : 4 * k4 + 4, b_ * TKV : (b_ + 1) * TKV
                            ],
                        )

                def proj_mm(ps, w_sb, x_in):
                    for kc2 in range(D // 256):
                        nc.tensor.matmul(
                            ps,
                            w_sb[:, 2 * kc2 : 2 * kc2 + 2, :],
                            x_in[:, 2 * kc2 : 2 * kc2 + 2, :],
                            start=(kc2 == 0),
                            stop=(kc2 == D // 256 - 1),
                            perf_mode=DR,
                        )

                def emit_q_chunk(j):
                    ps = pps.tile([FPC, CH], f32, tag="ps", bufs=1, name=f"ps_q{j}")
                    proj_mm(ps, wq_sb, xq_sb[:, :, j * CH : (j + 1) * CH])
                    sl = slice(j * CH, (j + 1) * CH)
                    for h_ in range(2):
                        hs = slice(64 * h_, 64 * h_ + 64)
                        nc.vector.tensor_scalar(
                            out=qcb_sb[hs, sl, h_], in0=ps[hs, :],
                            scalar1=1.0 / WS, scalar2=cb_sb[hs, :],
                            op0=ALU.mult, op1=ALU.add,
                        )
                        nc.vector.tensor_scalar(
                            out=qpb_sb[hs, sl, h_], in0=ps[hs, :],
                            scalar1=1.0 / WS, scalar2=pb_sb[hs, :],
                            op0=ALU.mult, op1=ALU.add,
                        )

                def emit_kvr_chunk(j):
                    b_, jj = j // 4, j % 4
                    kv_in = xkv_h[b_][:, :, jj * CH : (jj + 1) * CH]
                    sl = slice(j * CH, (j + 1) * CH)
                    ps = pps.tile([FPC, CH], f32, tag="ps", bufs=1, name=f"ps_k{j}")
                    proj_mm(ps, wk_sb, kv_in)
                    nc.vector.tensor_scalar_mul(
                        out=kT_sb[:, sl], in0=ps, scalar1=1.0 / WS
                    )
                    # v: feature-major, de-scale to bf16, then pair-major
                    # PE transposes into fp8 v_pm
                    psvt = pps.tile([FPC, CH], f32, tag="ps", bufs=1, name=f"psvt{j}")
                    proj_mm(psvt, wv_sb, kv_in)
                    vt_t = pa_vt.tile([FPC, CH], bf16, tag="vt_t", name=f"vt_t{j}")
                    nc.vector.tensor_scalar_mul(out=vt_t, in0=psvt, scalar1=1.0 / WS)
                    for w2 in range(CH // 256):
                        for par in range(2):
                            psv = pps.tile(
                                [128, FPC], bf16, tag="psv", bufs=1,
                                name=f"psv{j}_{w2}_{par}"
                            )
                            nc.tensor.transpose(
                                psv,
                                vt_t[:, 256 * w2 + par : 256 * (w2 + 1) : 2],
                                ident_bf,
                            )
                            if par == 0:
                                nc.scalar.copy(
                                    out=v_pm[:, j * 2 + w2, par, :], in_=psv
                                )
                            else:
                                nc.vector.tensor_copy(
                                    out=v_pm[:, j * 2 + w2, par, :], in_=psv
                                )
                    ps2 = pps.tile([FPC, CH], f32, tag="ps", bufs=1, name=f"ps_r{j}")
                    proj_mm(ps2, wr_sb, xr_h[b_][:, :, jj * CH : (jj + 1) * CH])
                    nc.vector.tensor_scalar_mul(
                        out=rT_sb[:, sl], in0=ps2, scalar1=1.0 / WS
                    )

                def emit_pass1(sp, hf):
                    # dual-tiles t2 in this q-half: 64 q-rows per head,
                    # both heads computed by one K=128 block-diag matmul
                    b = sp
                    for tl2 in range(8):
                        t2 = 8 * hf + tl2
                        c0 = max(0, (960 - 64 * t2) // 128 * 128)
                        bd_row = pb_rows.tile([128, TKV], fp8, tag="bd_row")
                        col = c0
                        while col < TKV:
                            cw = min(512 - col % 512, TKV - col)
                            ps_bd = pps.tile([128, 512], f32, tag="ps_sc", bufs=3)
                            nc.tensor.matmul(
                                ps_bd[:, :cw],
                                qpb_sb[:, b * TQ + 64 * t2 : b * TQ + 64 * t2 + 64, :],
                                rT_sb[:, b * TKV + col : b * TKV + col + cw],
                                start=True,
                                stop=True,
                            )
                            nc.vector.tensor_copy(
                                out=bd_row[:, col : col + cw], in_=ps_bd[:, :cw]
                            )
                            col += cw
                        # bd raw writes are latency-tolerant (consumed a full
                        # hf-phase later): SWDGE queue, keeping the scalar
                        # HWDGE free for the latency-critical shifted reads
                        # and attn transposes
                        nc.gpsimd.dma_start(
                            out=AP(
                                tensor=bd_dram[sp][hf][:].tensor,
                                offset=tl2 * 64 * TKV + c0,
                                ap=[[TKV, 64], [512 * TKV, 2], [1, TKV - c0]],
                            ),
                            in_=bd_row[:, c0:TKV],
                        )

                def emit_pass2(sp, hf):
                    b = sp
                    rmat = pb_recip.tile(
                        [128, 8], f32, tag="rmat", name=f"rm{sp}_{hf}"
                    )
                    for tl2 in range(8):
                        t2 = 8 * hf + tl2
                        ncc = 1152 + 128 * (t2 // 2)  # causal cols, 128-aligned
                        nc512 = 1536 if t2 < 8 else TKV
                        bd_shift = pb_rows.tile([128, TKV], fp8, tag="bd_shift")
                        nc.sync.dma_start(
                            out=bd_shift[:, :ncc],
                            in_=AP(
                                tensor=bd_dram[sp][hf][:].tensor,
                                offset=tl2 * 64 * TKV
                                + (TQ - 1)
                                - 512 * hf
                                - 64 * tl2,
                                ap=[[TKV - 1, 64], [512 * TKV, 2], [1, ncc]],
                            ),
                        )
                        # select iff 2*base0 + p - 2*j >= 0, i.e.
                        # j <= base0 + p//2 (p = 2c+h, both heads share c)
                        nc.gpsimd.affine_select(
                            out=bd_shift[:, ncc - 128 : ncc],
                            in_=bd_shift[:, ncc - 128 : ncc],
                            pattern=[[-2, 128]],
                            compare_op=ALU.is_ge,
                            fill=fill_reg,
                            base=128 * (t2 % 2),
                            channel_multiplier=1,
                        )
                        attn_row = pb_sum.tile([128, TKV], fp8, tag="attn_row")
                        if ncc < nc512:
                            nc.gpsimd.memset(attn_row[:, ncc:nc512], 0.0)
                        # ac+bd summed ON the PE: the ac matmul opens the PSUM
                        # group and an fp8 identity matmul accumulates the
                        # shifted bd on top; exp then reads the PSUM bank
                        # directly (per chunk, partial row sums combined on
                        # DVE). This removes the DVE tensor_add from the
                        # ac->exp pacing chain entirely.
                        nch = (ncc + 511) // 512
                        rs4 = pb_small.tile([128, 4], f32, tag="rs4")
                        for n in range(nch):
                            cw = min(512, ncc - 512 * n)
                            ps_ac = pps.tile([128, 512], f32, tag="ps_ac", bufs=3)
                            nc.tensor.matmul(
                                ps_ac[:, :cw],
                                qcb_sb[:, b * TQ + 64 * t2 : b * TQ + 64 * t2 + 64, :],
                                kT_sb[:, b * TKV + 512 * n : b * TKV + 512 * n + cw],
                                start=True,
                                stop=False,
                            )
                            nc.tensor.matmul(
                                ps_ac[:, :cw],
                                ident_f8,
                                bd_shift[:, 512 * n : 512 * n + cw],
                                start=False,
                                stop=True,
                            )
                            nc.scalar.activation(
                                out=attn_row[:, 512 * n : 512 * n + cw],
                                in_=ps_ac[:, :cw],
                                func=Exp,
                                scale=0.125,
                                accum_out=rs4[:, n : n + 1],
                            )
                        rowsum = pb_small.tile([128, 1], f32, tag="rowsum")
                        nc.vector.tensor_reduce(
                            out=rowsum, in_=rs4[:, :nch], axis=AXL.X, op=ALU.add
                        )
                        recip = pb_small.tile([128, 1], f32, tag="recip")
                        nc.vector.reciprocal(recip, rowsum)
                        nc.vector.tensor_scalar_mul(
                            out=rmat[:, tl2 : tl2 + 1], in0=recip, scalar1=CTX_S
                        )
                        nc.gpsimd.dma_start(
                            out=AP(
                                tensor=attn_dram[sp][hf][:].tensor,
                                offset=64 * tl2 * (TKV // 2),
                                ap=[
                                    [TKV // 2, 64],
                                    [512 * (TKV // 2), 2],
                                    [1, nc512 // 2],
                                ],
                            ),
                            in_=attn_row[:, :nc512].bitcast(u16),
                        )
                    # export recip: flat interleaved layout (2*q_local + head)
                    nc.sync.dma_start(
                        out=AP(
                            tensor=recip_dram[sp][hf][:].tensor,
                            offset=0,
                            ap=[[1, 128], [128, 8]],
                        ),
                        in_=rmat,
                    )

                ctx_ps = {}

                def emit_ctx_nn(sp, hh, n_):
                    b = sp
                    qf = slice(64 * hh, 64 * hh + 64)
                    rb = pb_t.tile(
                        [64, 1024], f32, tag="recip_bc", name=f"rb{sp}_{hh}_{n_}"
                    )
                    nc.sync.dma_start(
                        out=rb,
                        in_=AP(
                            tensor=recip_dram[sp][n_][:].tensor,
                            offset=0,
                            ap=[[0, 64], [1, 1024]],
                        ),
                    )
                    ps_cx = pps.tile(
                        [64, 512], f32, tag="psv", bufs=1,
                        name=f"ps_cx{sp}_{hh}_{n_}"
                    )
                    nws = 6 if n_ == 0 else NW
                    # ONE region transpose instead of one per 128-col window:
                    # xbar out [p, w, q] extends the partition dim with the
                    # extra dim, so source col 128w+p lands at partition p,
                    # plane w - exactly the per-window layout, 8 instructions
                    # instead of 56 on the hwdge queues
                    atr = pb_t.tile(
                        [128, NW, 512], u16, tag="attnT2b", bufs=2,
                        name=f"atr{sp}_{hh}_{n_}"
                    )
                    eng = nc.sync if hh == 0 else nc.scalar
                    eng.dma_start(
                        out=atr[:, :nws, :],
                        in_=attn_dram[sp][n_][
                            512 * hh : 512 * hh + 512, 0 : 128 * nws
                        ],
                        transpose=True,
                    )
                    for w in range(nws):
                        rr = (
                            atr[:, w, :]
                            .bitcast(fp8)
                            .rearrange("p (q i) -> p i q", i=2)
                        )
                        nc.tensor.matmul(
                            ps_cx,
                            v_pm[:, b * NW + w, :, qf],
                            rr,
                            start=(w == 0),
                            stop=(w == nws - 1),
                            perf_mode=DR,
                        )
                    nc.vector.tensor_mul(
                        out=ctx_sb[qf, b * TQ + 512 * n_ : b * TQ + 512 * (n_ + 1)],
                        in0=ps_cx,
                        in1=rb[:, hh::2],
                    )

                def emit_a2a(bc):
                    nc.sync.dma_start(
                        out=a2a_in[bc][:].rearrange("(j p) t -> p j t", p=FPC),
                        in_=ctx_sb[:, bc * TQ : (bc + 1) * TQ].rearrange(
                            "p (j t) -> p j t", t=TQ // 8
                        ),
                    )
                    if n_cores > 1:
                        nc.gpsimd.collective_compute(
                            "AllToAll",
                            ALU.bypass,
                            replica_groups=[list(range(n_cores))],
                            ins=[a2a_in[bc][:]],
                            outs=[a2a_out[bc][:]],
                        )
                    else:
                        nc.sync.dma_start(out=a2a_out[bc][:], in_=a2a_in[bc][:])

                def emit_phase_c(bc):
                    ps_o = [
                        pps.tile([128, 512], f32, tag="ps_sc", bufs=3, name=f"ps_o{bc}_{nn_}")
                        for nn_ in range(2)
                    ]
                    for kc2 in range(D // 256):
                        lhs = pc.tile([128, 2, 128], fp8, tag="octx")
                        nc.sync.dma_start(
                            out=lhs,
                            in_=a2a_out[bc][
                                kc2 * 256 : (kc2 + 1) * 256, :
                            ].rearrange("(i p) t -> p i t", p=128),
                        )
                        for nn in range(2):
                            nc.tensor.matmul(
                                ps_o[nn],
                                lhs,
                                wo_sb[:, 2 * kc2 : 2 * kc2 + 2, nn * 512 : (nn + 1) * 512],
                                start=(kc2 == 0),
                                stop=(kc2 == D // 256 - 1),
                                perf_mode=DR,
                            )
                    o_sb = pc.tile([128, D], f32, tag="o_sb")
                    for nn in range(2):
                        nc.vector.tensor_add(
                            out=o_sb[:, nn * 512 : (nn + 1) * 512],
                            in0=ps_o[nn],
                            in1=qres_sb[:, bc, nn * 512 : (nn + 1) * 512],
                        )
                    # LayerNorm over the free (feature) dim
                    stats = pc_small.tile([128, 2, 6], f32, tag="stats")
                    for sg in range(2):
                        nc.vector.bn_stats(
                            out=stats[:, sg, :], in_=o_sb[:, sg * 512 : (sg + 1) * 512]
                        )
                    mv = pc_small.tile([128, 2], f32, tag="mv")
                    nc.vector.bn_aggr(out=mv, in_=stats)
                    mean, var = mv[:, 0:1], mv[:, 1:2]
                    xve = pc_small.tile([128, 1], f32, tag="xve")
                    nc.vector.tensor_scalar_add(out=xve, in0=var, scalar1=eps_sb)
                    # rsqrt = exp(-0.5*ln(var+eps)): Ln+Exp live in the same
                    # activation table as the attention Exp (Sqrt does not,
                    # and would force a 1.3us table reload each LayerNorm)
                    lnv = pc_small.tile([128, 1], f32, tag="lnv")
                    nc.scalar.activation(out=lnv, in_=var, func=Ln, bias=eps_sb)
                    rstd = pc_small.tile([128, 1], f32, tag="rstd")
                    nc.scalar.activation(out=rstd, in_=lnv, func=Exp, scale=-0.5)
                    # one Newton step for rsqrt accuracy:
                    # r <- r * (1.5 - 0.5 * x * r^2)
                    tnw = pc_small.tile([128, 1], f32, tag="tnw")
                    nc.vector.tensor_mul(out=tnw, in0=rstd, in1=rstd)
                    nc.vector.tensor_mul(out=tnw, in0=tnw, in1=xve)
                    nc.vector.tensor_scalar(
                        out=tnw, in0=tnw, scalar1=-0.5, scalar2=1.5,
                        op0=ALU.mult, op1=ALU.add,
                    )
                    nc.vector.tensor_scalar_mul(out=rstd, in0=rstd, scalar1=tnw)
                    nc.vector.tensor_scalar(
                        out=o_sb, in0=o_sb, scalar1=mean, scalar2=rstd,
                        op0=ALU.subtract, op1=ALU.mult,
                    )
                    nc.vector.tensor_mul(out=o_sb, in0=o_sb, in1=gamma_sb)
                    nc.vector.tensor_add(out=o_sb, in0=o_sb, in1=beta_sb)
                    nc.sync.dma_start(
                        out=out[bc * 128 : (bc + 1) * 128, :], in_=o_sb
                    )

                for j in range(4):
                    emit_q_chunk(j)
                for j in range(8):
                    emit_kvr_chunk(j)
                pin.release()
                pa_vt.release()
                # output-phase constants load into the space freed by the raw
                # activations, during the attention phase
                late = tc.alloc_tile_pool(name="late", bufs=1)
                wo_sb = late.tile([128, D // 128, D], fp8, tag="wo_sb")
                nc.gpsimd.dma_start(out=wo_sb, in_=wo[:])
                gamma_sb = late.tile([128, D], f32, tag="gamma_sb")
                beta_sb = late.tile([128, D], f32, tag="beta_sb")
                nc.gpsimd.dma_start(
                    out=gamma_sb,
                    in_=AP(tensor=gamma[:].tensor, offset=0, ap=[[0, 128], [1, D]]),
                )
                nc.gpsimd.dma_start(
                    out=beta_sb,
                    in_=AP(tensor=beta[:].tensor, offset=0, ap=[[0, 128], [1, D]]),
                )
                qres_sb = late.tile([128, RPC // 128, D], f32, tag="qres_sb")
                nc.gpsimd.dma_start(
                    out=qres_sb, in_=qres[:].rearrange("(mc p) d -> p mc d", p=128)
                )
                # zero-init bd scratch regions the shifted reads can touch but
                # raw writes never cover (wrap into the next row's low cols)
                zeros_f8 = pb_t.tile([128, 2048], fp8, tag="zeros_f8")
                nc.vector.memset(zeros_f8, 0.0)

                def zero_scratch(sp):
                    # pre-zero so any racy early read sees 0.0, never an fp8
                    # NaN byte (uninitialized DRAM): masked positions are
                    # replaced downstream, unmasked races stay tiny+finite
                    for hf in range(2):
                        for blk in range(8):
                            nc.sync.dma_start(
                                out=AP(
                                    tensor=bd_dram[sp][hf][:].tensor,
                                    offset=blk * 128 * TKV,
                                    ap=[[TKV, 128], [1, TKV]],
                                ),
                                in_=zeros_f8,
                            )
                    if sp == 0:
                        for bc in range(2):
                            nc.gpsimd.dma_start(
                                out=a2a_out[bc][:].rearrange(
                                    "(j p) t -> p j t", p=FPC
                                ),
                                in_=zeros_f8[:, : TQ // 8 * 8].rearrange(
                                    "p (j t) -> p j t", j=8
                                ),
                            )

                # interleaved emission: independent PE work between the
                # dependent stages. The AllToAll triggers (emit_a2a) are
                # decoupled from their consumers (emit_phase_c) so the PE's
                # in-order queue is never head-of-line blocked waiting on a
                # collective: both output projections run at the end, with
                # batch-0's overlapping batch-1's AllToAll latency.
                # hf=1 (the wide-causal half) runs FIRST within each batch so
                # the drain tail is the cheap half; each ctx pair follows its
                # producing pass2 immediately so attn-transpose chains overlap
                # the other half's score work
                # zero_scratch is NOT emitted: internal DRAM is
                # zero-initialized at NEFF load, and the 4MB of zero writes
                # were delaying the sync-queue attn transposes behind them
                emit_pass1(0, 1)
                emit_pass2(0, 1)
                emit_pass1(0, 0)
                emit_pass2(0, 0)
                emit_ctx_nn(0, 0, 1)
                emit_ctx_nn(0, 1, 1)
                emit_pass1(1, 1)
                emit_ctx_nn(0, 0, 0)
                emit_ctx_nn(0, 1, 0)
                emit_a2a(0)
                emit_pass2(1, 1)
                emit_pass1(1, 0)
                emit_pass2(1, 0)
                emit_ctx_nn(1, 0, 1)
                emit_ctx_nn(1, 1, 1)
                emit_ctx_nn(1, 0, 0)
                emit_ctx_nn(1, 1, 0)
                emit_a2a(1)
                # schedule the output projections LAST: the tile scheduler's
                # cost model thinks the AllToAll is fast and would otherwise
                # hoist the Wo matmuls in front of ready attention work,
                # head-of-line blocking the PE on the collective for ~35us
                with tc.tile_wait_until(1.0):
                    emit_phase_c(0)
                with tc.tile_wait_until(1.1):
                    emit_phase_c(1)
                late.release()
                pps.release()
    return nc


def _make_in_maps(inputs, mm_dtype="float8e4"):
    import ml_dtypes

    f8 = ml_dtypes.float8_e4m3

    query = np.asarray(inputs["query"], np.float32)
    key_value = np.asarray(inputs["key_value"], np.float32)
    relative = np.asarray(inputs["relative"], np.float32)
    content_bias = np.asarray(inputs["content_bias"], np.float32)
    position_bias = np.asarray(inputs["position_bias"], np.float32)
    Wq, Wk = np.asarray(inputs["Wq"], np.float32), np.asarray(inputs["Wk"], np.float32)
    Wv, Wr = np.asarray(inputs["Wv"], np.float32), np.asarray(inputs["Wr"], np.float32)
    Wo = np.ascontiguousarray(np.asarray(inputs["Wo"], np.float32))
    ln_gamma = np.asarray(inputs["ln_gamma"], np.float32)
    ln_beta = np.asarray(inputs["ln_beta"], np.float32)

    qflat = query.reshape(B * TQ, D)
    xqT = np.ascontiguousarray(qflat.T).astype(f8)
    xkvT = np.ascontiguousarray(key_value.reshape(B * TKV, D).T).astype(f8)
    xrT = np.ascontiguousarray(relative.reshape(B * TKV, D).T).astype(f8)
    Wq8 = (Wq * WS).astype(f8)
    Wk8 = (Wk * WS).astype(f8)
    Wv8 = (Wv * WS).astype(f8)
    Wr8 = (Wr * WS).astype(f8)
    Wo8 = (Wo * WS).astype(f8)
    cb = content_bias.reshape(NH, DV)
    pb = position_bias.reshape(NH, DV)

    def parr(w):  # [D, F] -> [p, kc, F]: per-partition rows for fat descriptors
        return np.ascontiguousarray(
            w.reshape(D // 128, 128, w.shape[1]).transpose(1, 0, 2)
        )

    in_maps = []
    for c in range(N_CORES):
        fs = slice(FPC * c, FPC * (c + 1))
        # out rows: [0,128) = batch-0 tokens [128c,+128),
        #           [128,256) = batch-1 tokens [128c,+128)
        qr = np.concatenate(
            [
                qflat[128 * c : 128 * c + 128],
                qflat[TQ + 128 * c : TQ + 128 * c + 128],
            ]
        )
        in_maps.append(
            {
                "xqT": xqT,
                "xkvT": xkvT,
                "xrT": xrT,
                "wq": parr(Wq8[:, fs]),
                "wk": parr(Wk8[:, fs]),
                "wv": parr(Wv8[:, fs]),
                "wr": parr(Wr8[:, fs]),
                "wo": parr(Wo8),
                "cbv": np.ascontiguousarray(
                    cb[HPC * c : HPC * (c + 1)].reshape(FPC, 1)
                ),
                "pbv": np.ascontiguousarray(
                    pb[HPC * c : HPC * (c + 1)].reshape(FPC, 1)
                ),
                "qres": np.ascontiguousarray(qr) * RES_S,
                "gamma": ln_gamma,
                "beta": ln_beta,
            }
        )
    return in_maps


def run_on_hw(inputs, trace=False, score_dtype="bfloat16", proj_dtype="float8e4"):
    from concourse.bass_utils import run_bass_kernel_spmd

    key = ("v3",)
    nc = _CACHE.get(key)
    if nc is None:
        nc = build_program()
        _CACHE[key] = nc
    in_maps = _make_in_maps(inputs)
    res = run_bass_kernel_spmd(nc, in_maps, list(range(N_CORES)), trace=trace)
    # core c rows: [b*128, b*128+128) = batch b tokens [128c, 128c+128)
    outs = np.stack(
        [np.asarray(res.results[c]["out"]) for c in range(N_CORES)]
    )  # [8, 256, D]
    outs = outs.reshape(N_CORES, B, 128, D).transpose(1, 0, 2, 3)
    return np.ascontiguousarray(outs.reshape(B, TQ, D)), res


def kernel(**inputs) -> np.ndarray:
    out, _ = run_on_hw(inputs)
    return out

